# revision 23
# baseline (speedup 1.0000x reference)
"""HardClusterAssigner Trainium2 kernel.

Reference computation:
    x_emb = mean_b(einsum('bsv,hs->bvh', x, W) + b)   # [V, H]
    assignments = one_hot(argmin(-l2norm(x_emb) @ l2norm(centroids).T))

Key transformations used here:
  1. mean over B commutes with the (linear) contraction over S:
         mean_b(x @ W.T) = (mean_b x) @ W.T
     so the 34-GFLOP batched matmul collapses to a memory-bound reduction
     of x over B followed by one [V,S]x[S,H] matmul.
  2. l2norm of the embedding is a positive per-row scale -> it cannot change
     the row-wise argmin, so it is skipped. Only centroids need normalizing.
  3. The 1/B mean scale and the bias are folded in exactly:
         B * (mean_b(xW.T) + bias) = (sum_b x) @ W.T + B*bias
     and the overall positive factor B is again argmin-invariant.

Sharding: V (last dim of x) is split across the 8 cores; every stage after
the split is core-local (no collectives). Each core computes its 64 rows of
the one-hot output.
"""

import sys

for _p in ("/opt/trn_rl_repo",):
    if _p not in sys.path:
        sys.path.append(_p)

from contextlib import ExitStack

import numpy as np

import concourse.bacc as bacc
import concourse.bass as bass
import concourse.mybir as mybir
from concourse import tile
from concourse.bass_utils import run_bass_kernel_spmd
from concourse.masks import make_identity

B, S, V, H, C = 64, 1024, 512, 512, 64
NCORES = 8
VL = V // NCORES  # 64 V-columns per core
P = 128
ST = S // P  # 8 s-chunks
F32 = mybir.dt.float32

_NC_CACHE = None


def build_bass() -> bass.Bass:
    nc = bacc.Bacc("TRN2", target_bir_lowering=False)

    xs = nc.declare_dram_parameter("xs", [S, VL, B], F32, isOutput=False)
    wt = nc.declare_dram_parameter("wt", [P, 4 * ST * P], F32, isOutput=False)
    bb = nc.declare_dram_parameter("bb", [H, 1], F32, isOutput=False)
    cent = nc.declare_dram_parameter("cent", [C, H], F32, isOutput=False)
    out = nc.declare_dram_parameter("out", [VL, C], F32, isOutput=True)

    with tile.TileContext(nc) as tc, ExitStack() as ctx:
        consts = ctx.enter_context(tc.tile_pool(name="consts", bufs=1))
        xpool = ctx.enter_context(tc.tile_pool(name="x", bufs=16))
        xmpool = ctx.enter_context(tc.tile_pool(name="xm", bufs=1))
        spool = ctx.enter_context(tc.tile_pool(name="small", bufs=1))
        psum = ctx.enter_context(tc.tile_pool(name="psum", bufs=1, space="PSUM"))
        tpsum = ctx.enter_context(tc.tile_pool(name="tpsum", bufs=2, space="PSUM"))

        # --- constants / small inputs -------------------------------------
        # const DMAs ride the ACT HWDGE ring so x tiles own the SP ring;
        # centroids first (needed by the early normalize), W last.
        centt = spool.tile([C, H], F32)
        nc.scalar.dma_start(out=centt[:], in_=cent[:])
        bbt = consts.tile([P, 4], F32)  # B*b as column chunks: h = k*128 + p
        nc.scalar.dma_start(out=bbt[:], in_=bb.rearrange("(k p) o -> p k o", p=P))
        # W pre-tiled on host to [p, hk, t, q] so this DMA is fully contiguous
        wsb = consts.tile([P, 4, ST, P], F32)
        nc.scalar.dma_start(
            out=wsb[:], in_=wt.rearrange("p (hk t q) -> p hk t q", hk=4, t=ST)
        )

        ones_row = consts.tile([1, VL], F32)
        nc.vector.memset(ones_row[:], 1.0)

        ident = consts.tile([P, P], F32)
        make_identity(nc, ident[:])

        # centroid row norms: square+row-sum fused on ACT (cheap, early)
        csq = spool.tile([C, H], F32)
        ssq = spool.tile([C, 1], F32)
        nc.scalar.activation(
            csq[:], centt[:], mybir.ActivationFunctionType.Square, accum_out=ssq[:]
        )
        cnorm = spool.tile([C, 1], F32)
        nc.scalar.sqrt(cnorm[:], ssq[:])
        cinv = spool.tile([C, 1], F32)
        nc.vector.reciprocal(cinv[:], cnorm[:])
        centn = spool.tile([C, H], F32)
        nc.vector.tensor_scalar_mul(centn[:], centt[:], cinv[:])

        # cnT: normalized centroids transposed to [H, C] chunks
        cenT = spool.tile([P, 4 * C], F32)
        for k in range(4):
            cp = tpsum.tile([P, C], F32, tag="tp")
            nc.tensor.transpose(cp[:], centn[:, k * P : (k + 1) * P], ident[:C, :C])
            nc.scalar.copy(cenT[:, k * C : (k + 1) * C], cp[:])

        # bias row in sim space: b_n[c] = sum_h (B*b)[h] * cn[c, h]
        bn_ps = psum.tile([1, C], F32, tag="bn")
        for k in range(4):
            nc.tensor.matmul(
                bn_ps[:],
                bbt[:, k : k + 1],
                cenT[:, k * C : (k + 1) * C],
                start=(k == 0),
                stop=(k == 3),
            )
        bn_sb = spool.tile([1, C], F32)
        nc.scalar.copy(bn_sb[:], bn_ps[:])

        # --- x stream: DMA + reduce over B + per-chunk sim matmul ---------
        # sim[v,c] = sum_t xm_t[s,v]^T (W_t @ cnT)[s,c] + ones^T b_n
        # xs[s, v, b]; tile t holds s in [t*128, (t+1)*128); b innermost so
        # the reduce streams unit-stride. Two v-halves per s-chunk (1MiB
        # DMAs) for finer DMA/DVE pipelining.
        HV = VL // 2  # 32
        xs_r = xs.rearrange("(t p) v b -> t p (v b)", p=P)
        sim_ps = psum.tile([VL, C], F32, tag="sim")
        nc.tensor.matmul(sim_ps[:], ones_row[:], bn_sb[:], start=True, stop=False)
        for t in range(ST):
            # Mt = W_t @ cnT : [128 s, 64 c], overlapped with the x stream
            mt_ps = tpsum.tile([P, C], F32, tag="mt")
            for hk in range(4):
                nc.tensor.matmul(
                    mt_ps[:],
                    wsb[:, hk, t, :],
                    cenT[:, hk * C : (hk + 1) * C],
                    start=(hk == 0),
                    stop=(hk == 3),
                )
            mt_sb = spool.tile([P, C], F32, tag=f"mt{t}")
            nc.scalar.copy(mt_sb[:], mt_ps[:])

            xm = xmpool.tile([P, VL], F32, tag=f"xm{t}")
            for h in range(2):
                xt = xpool.tile([P, HV * B], F32, tag="xt")
                nc.sync.dma_start(
                    out=xt[:], in_=xs_r[t][:, h * HV * B : (h + 1) * HV * B]
                )
                nc.vector.tensor_reduce(
                    xm[:, h * HV : (h + 1) * HV],
                    xt[:].rearrange("p (v b) -> p v b", b=B),
                    axis=mybir.AxisListType.X,
                    op=mybir.AluOpType.add,
                )
            nc.tensor.matmul(
                sim_ps[:], xm[:], mt_sb[:], start=False, stop=(t == ST - 1)
            )

        # --- one-hot of row argmax ----------------------------------------
        mx = spool.tile([VL, 1], F32)
        nc.vector.tensor_reduce(
            mx[:], sim_ps[:], axis=mybir.AxisListType.X, op=mybir.AluOpType.max
        )
        oh = spool.tile([VL, C], F32)
        nc.vector.tensor_scalar(
            oh[:], sim_ps[:], mx[:], None, op0=mybir.AluOpType.is_equal
        )
        nc.sync.dma_start(out=out[:], in_=oh[:])

    nc.compile()
    return nc


def _get_nc() -> bass.Bass:
    global _NC_CACHE
    if _NC_CACHE is None:
        _NC_CACHE = build_bass()
    return _NC_CACHE


def make_in_maps(x, W, b, centroids):
    x = np.asarray(x, dtype=np.float32)
    W = np.asarray(W, dtype=np.float32)
    b = np.asarray(b, dtype=np.float32)
    centroids = np.asarray(centroids, dtype=np.float32)

    # W[hk*128+p, t*128+q] -> [p, (hk, t, q)] so the device DMA is contiguous
    wt_host = np.ascontiguousarray(
        W.reshape(4, P, ST, P).transpose(1, 0, 2, 3)
    ).reshape(P, 4 * ST * P)
    brow = (np.float32(B) * b).reshape(H, 1).astype(np.float32)
    cent_host = np.ascontiguousarray(centroids)

    # Two-step host transpose [B,S,V] -> [S,V,B]: one pass to [S,B,V]
    # (contiguous 2KB runs, fast), then per-s [B,VL] -> [VL,B] blocks that
    # stay cache-resident. Direct one-shot transpose would thrash DRAM.
    xsb = np.ascontiguousarray(x.transpose(1, 0, 2))  # [S, B, V]
    in_maps = []
    for i in range(NCORES):
        xs_i = np.ascontiguousarray(
            xsb[:, :, i * VL : (i + 1) * VL].transpose(0, 2, 1)
        )  # [S, VL, B]
        in_maps.append({"xs": xs_i, "wt": wt_host, "bb": brow, "cent": cent_host})
    return in_maps


def run(inputs: dict, trace: bool = False):
    """Run on the 8 NeuronCores; returns (full_output, BassKernelResults)."""
    nc = _get_nc()
    in_maps = make_in_maps(**inputs)
    res = run_bass_kernel_spmd(nc, in_maps, list(range(NCORES)), trace=trace)
    full = np.concatenate([r["out"] for r in res.results], axis=0)
    return full, res


def kernel(x, W, b, centroids) -> np.ndarray:
    full, _ = run({"x": x, "W": W, "b": b, "centroids": centroids})
    return full


# revision 24
# speedup vs baseline: 1.0459x; 1.0459x over previous
"""HardClusterAssigner Trainium2 kernel.

Reference computation:
    x_emb = mean_b(einsum('bsv,hs->bvh', x, W) + b)   # [V, H]
    assignments = one_hot(argmin(-l2norm(x_emb) @ l2norm(centroids).T))

Key transformations used here:
  1. mean over B commutes with the (linear) contraction over S:
         mean_b(x @ W.T) = (mean_b x) @ W.T
     so the 34-GFLOP batched matmul collapses to a memory-bound reduction
     of x over B followed by one [V,S]x[S,H] matmul.
  2. l2norm of the embedding is a positive per-row scale -> it cannot change
     the row-wise argmin, so it is skipped. Only centroids need normalizing.
  3. The 1/B mean scale and the bias are folded in exactly:
         B * (mean_b(xW.T) + bias) = (sum_b x) @ W.T + B*bias
     and the overall positive factor B is again argmin-invariant.

Sharding: V (last dim of x) is split across the 8 cores; every stage after
the split is core-local (no collectives). Each core computes its 64 rows of
the one-hot output.
"""

import sys

for _p in ("/opt/trn_rl_repo",):
    if _p not in sys.path:
        sys.path.append(_p)

from contextlib import ExitStack

import numpy as np

import concourse.bacc as bacc
import concourse.bass as bass
import concourse.mybir as mybir
from concourse import tile
from concourse.bass_utils import run_bass_kernel_spmd
from concourse.masks import make_identity

B, S, V, H, C = 64, 1024, 512, 512, 64
NCORES = 8
VL = V // NCORES  # 64 V-columns per core
P = 128
ST = S // P  # 8 s-chunks
F32 = mybir.dt.float32

_NC_CACHE = None


def build_bass() -> bass.Bass:
    nc = bacc.Bacc("TRN2", target_bir_lowering=False)

    xs = nc.declare_dram_parameter("xs", [S, VL, B], F32, isOutput=False)
    wt = nc.declare_dram_parameter("wt", [P, 4 * ST * P], F32, isOutput=False)
    bb = nc.declare_dram_parameter("bb", [H, 1], F32, isOutput=False)
    cent = nc.declare_dram_parameter("cent", [C, H], F32, isOutput=False)
    out = nc.declare_dram_parameter("out", [VL, C], F32, isOutput=True)

    with tile.TileContext(nc) as tc, ExitStack() as ctx:
        consts = ctx.enter_context(tc.tile_pool(name="consts", bufs=1))
        xpool = ctx.enter_context(tc.tile_pool(name="x", bufs=12))
        xmpool = ctx.enter_context(tc.tile_pool(name="xm", bufs=1))
        spool = ctx.enter_context(tc.tile_pool(name="small", bufs=1))
        psum = ctx.enter_context(tc.tile_pool(name="psum", bufs=1, space="PSUM"))
        tpsum = ctx.enter_context(tc.tile_pool(name="tpsum", bufs=2, space="PSUM"))

        # --- constants / small inputs -------------------------------------
        # const DMAs ride the ACT HWDGE ring so x tiles own the SP ring;
        # centroids first (needed by the early normalize), W last.
        centt = spool.tile([C, H], F32)
        nc.scalar.dma_start(out=centt[:], in_=cent[:])
        bbt = consts.tile([P, 4], F32)  # B*b as column chunks: h = k*128 + p
        nc.scalar.dma_start(out=bbt[:], in_=bb.rearrange("(k p) o -> p k o", p=P))
        # W pre-tiled on host to [p, hk, t, q] so this DMA is fully contiguous
        wsb = consts.tile([P, 4, ST, P], F32)
        nc.scalar.dma_start(
            out=wsb[:], in_=wt.rearrange("p (hk t q) -> p hk t q", hk=4, t=ST)
        )

        ones_row = consts.tile([1, VL], F32)
        nc.vector.memset(ones_row[:], 1.0)

        ident = consts.tile([P, P], F32)
        make_identity(nc, ident[:])

        # centroid row norms: square+row-sum fused on ACT (cheap, early)
        csq = spool.tile([C, H], F32)
        ssq = spool.tile([C, 1], F32)
        nc.scalar.activation(
            csq[:], centt[:], mybir.ActivationFunctionType.Square, accum_out=ssq[:]
        )
        cnorm = spool.tile([C, 1], F32)
        nc.scalar.sqrt(cnorm[:], ssq[:])
        cinv = spool.tile([C, 1], F32)
        nc.vector.reciprocal(cinv[:], cnorm[:])
        centn = spool.tile([C, H], F32)
        nc.vector.tensor_scalar_mul(centn[:], centt[:], cinv[:])

        # cnT: normalized centroids transposed to [H, C] chunks
        cenT = spool.tile([P, 4 * C], F32)
        for k in range(4):
            cp = tpsum.tile([P, C], F32, tag="tp")
            nc.tensor.transpose(cp[:], centn[:, k * P : (k + 1) * P], ident[:C, :C])
            nc.scalar.copy(cenT[:, k * C : (k + 1) * C], cp[:])

        # bias row in sim space: b_n[c] = sum_h (B*b)[h] * cn[c, h]
        bn_ps = psum.tile([1, C], F32, tag="bn")
        for k in range(4):
            nc.tensor.matmul(
                bn_ps[:],
                bbt[:, k : k + 1],
                cenT[:, k * C : (k + 1) * C],
                start=(k == 0),
                stop=(k == 3),
            )
        bn_sb = spool.tile([1, C], F32)
        nc.scalar.copy(bn_sb[:], bn_ps[:])

        # --- x stream: DMA + reduce over B + per-chunk sim matmul ---------
        # sim[v,c] = sum_t xm_t[s,v]^T (W_t @ cnT)[s,c] + ones^T b_n
        # xs[s, v, b]; tile t holds s in [t*128, (t+1)*128); b innermost so
        # the reduce streams unit-stride. Two v-halves per s-chunk (1MiB
        # DMAs) for finer DMA/DVE pipelining.
        HV = VL // 2  # 32
        xs_r = xs.rearrange("(t p) v b -> t p (v b)", p=P)
        sim_ps = psum.tile([VL, C], F32, tag="sim")
        nc.tensor.matmul(sim_ps[:], ones_row[:], bn_sb[:], start=True, stop=False)
        for t in range(ST):
            # Mt = W_t @ cnT : [128 s, 64 c], overlapped with the x stream
            mt_ps = tpsum.tile([P, C], F32, tag="mt")
            for hk in range(4):
                nc.tensor.matmul(
                    mt_ps[:],
                    wsb[:, hk, t, :],
                    cenT[:, hk * C : (hk + 1) * C],
                    start=(hk == 0),
                    stop=(hk == 3),
                )
            mt_sb = spool.tile([P, C], F32, tag=f"mt{t}")
            nc.scalar.copy(mt_sb[:], mt_ps[:])

            xm = xmpool.tile([P, VL], F32, tag=f"xm{t}")
            for h in range(2):
                xt = xpool.tile([P, HV * B], F32, tag="xt")
                nc.sync.dma_start(
                    out=xt[:], in_=xs_r[t][:, h * HV * B : (h + 1) * HV * B]
                )
                nc.vector.tensor_reduce(
                    xm[:, h * HV : (h + 1) * HV],
                    xt[:].rearrange("p (v b) -> p v b", b=B),
                    axis=mybir.AxisListType.X,
                    op=mybir.AluOpType.add,
                )
            nc.tensor.matmul(
                sim_ps[:], xm[:], mt_sb[:], start=False, stop=(t == ST - 1)
            )

        # --- one-hot of row argmax ----------------------------------------
        mx = spool.tile([VL, 1], F32)
        nc.vector.tensor_reduce(
            mx[:], sim_ps[:], axis=mybir.AxisListType.X, op=mybir.AluOpType.max
        )
        oh = spool.tile([VL, C], F32)
        nc.vector.tensor_scalar(
            oh[:], sim_ps[:], mx[:], None, op0=mybir.AluOpType.is_equal
        )
        nc.sync.dma_start(out=out[:], in_=oh[:])

    nc.compile()
    return nc


def _get_nc() -> bass.Bass:
    global _NC_CACHE
    if _NC_CACHE is None:
        _NC_CACHE = build_bass()
    return _NC_CACHE


def make_in_maps(x, W, b, centroids):
    x = np.asarray(x, dtype=np.float32)
    W = np.asarray(W, dtype=np.float32)
    b = np.asarray(b, dtype=np.float32)
    centroids = np.asarray(centroids, dtype=np.float32)

    # W[hk*128+p, t*128+q] -> [p, (hk, t, q)] so the device DMA is contiguous
    wt_host = np.ascontiguousarray(
        W.reshape(4, P, ST, P).transpose(1, 0, 2, 3)
    ).reshape(P, 4 * ST * P)
    brow = (np.float32(B) * b).reshape(H, 1).astype(np.float32)
    cent_host = np.ascontiguousarray(centroids)

    # Two-step host transpose [B,S,V] -> [S,V,B]: one pass to [S,B,V]
    # (contiguous 2KB runs, fast), then per-s [B,VL] -> [VL,B] blocks that
    # stay cache-resident. Direct one-shot transpose would thrash DRAM.
    xsb = np.ascontiguousarray(x.transpose(1, 0, 2))  # [S, B, V]
    in_maps = []
    for i in range(NCORES):
        xs_i = np.ascontiguousarray(
            xsb[:, :, i * VL : (i + 1) * VL].transpose(0, 2, 1)
        )  # [S, VL, B]
        in_maps.append({"xs": xs_i, "wt": wt_host, "bb": brow, "cent": cent_host})
    return in_maps


def run(inputs: dict, trace: bool = False):
    """Run on the 8 NeuronCores; returns (full_output, BassKernelResults)."""
    nc = _get_nc()
    in_maps = make_in_maps(**inputs)
    res = run_bass_kernel_spmd(nc, in_maps, list(range(NCORES)), trace=trace)
    full = np.concatenate([r["out"] for r in res.results], axis=0)
    return full, res


def kernel(x, W, b, centroids) -> np.ndarray:
    full, _ = run({"x": x, "W": W, "b": b, "centroids": centroids})
    return full
